# revision 1
# baseline (speedup 1.0000x reference)
"""AR(16) Gaussian log-likelihood kernel for Trainium2, 8 NeuronCores.

Math: out[b, t] = C - ((s[b,t] - sum_{k=1..16} phi_k s[b,t-k]) * invsc)^2
  with C = -0.5*log(2*pi*sigma^2), invsc = 1/(sqrt(2)*sigma).

Strategy (pure data parallel, 32 rows per core):
  - View each core's [32, 65536] shard as 8-row tiles laid out on 128
    SBUF partitions with U = 4096 contiguous f32 samples per partition,
    processed in half-tiles of H = 2048 samples (+32-sample halo).
  - Input DMA casts f32->bf16 (SWDGE); all 8 half-tile DMAs are issued
    up-front so the input queue streams continuously.
  - DVE stream-transposes the bf16 data elementwise: partition x of a
    32-group then holds single samples 32C + x, giving the matmul a
    stride-1 rhs and a 2-matrix banded-Toeplitz structure (dlt = 0 / -1).
  - TensorE computes z = (pred - s)*invsc in bf16 with 2 accumulating
    matmuls per 512-col PSUM bank at 4 diagonal tile positions (K=32).
  - ScalarE squares PSUM->SBUF (bf16), DVE applies C - x (4x bf16 mode).
  - The result is DMA'd out (bf16) still in the transposed block layout;
    the host unshard step de-interleaves it with a pure reshape/transpose
    and upcasts to f32. No on-chip back-transpose at all.
"""

import math

import numpy as np

import concourse.bass as bass
import concourse.tile as tile
from concourse import bacc, mybir
from concourse.bass_utils import run_bass_kernel_spmd

F32 = mybir.dt.float32
F32R = mybir.dt.float32r
BF16 = mybir.dt.bfloat16
U32 = mybir.dt.uint32
P = 16  # AR order
HALO = 32  # f32 halo = one 32-sample superblock

B_FULL, T_FULL = 256, 65536
N_CORES = 8

N_TOEP = 2  # dlt = 0, -1


def build_nc(b_core: int, t_len: int, rows_per_tile: int, win: int = 1024):
    R = rows_per_tile
    assert 128 % R == 0
    U = R * t_len // 128          # samples per partition per full tile
    cpr = 128 // R                # partitions per row
    assert cpr * U == t_len
    ntiles = b_core // R
    assert ntiles * R == b_core
    H = U // 2                    # half-tile samples per partition
    W = min(win, H)               # psum tile width (f32 columns)
    assert H % W == 0 and W % 512 == 0
    nwin = H // W
    nbank = W // 512              # 512-col psum banks per tile

    nc = bacc.Bacc(
        "TRN2", target_bir_lowering=False, debug=False, enable_asserts=False
    )
    s_h = nc.declare_dram_parameter("s", [b_core, t_len], F32, isOutput=False)
    toep_h = nc.declare_dram_parameter(
        "toep", [128, 32 * N_TOEP], BF16, isOutput=False
    )
    cvec_h = nc.declare_dram_parameter("cvec", [128, 1], F32, isOutput=False)
    halos_h = nc.declare_dram_parameter(
        "halos", [128, ntiles * HALO], F32, isOutput=False
    )
    out_h = nc.declare_dram_parameter(
        "out", [128, b_core * t_len // 128], BF16, isOutput=True
    )

    from contextlib import ExitStack

    with tile.TileContext(nc) as tc, ExitStack() as ctx:
        const_pool = ctx.enter_context(tc.tile_pool(name="const", bufs=1))
        in_pool = ctx.enter_context(tc.tile_pool(name="inp", bufs=8))
        raw_pool = ctx.enter_context(tc.tile_pool(name="raw", bufs=2))
        st_pool = ctx.enter_context(tc.tile_pool(name="stp", bufs=4))
        sq_pool = ctx.enter_context(tc.tile_pool(name="sqp", bufs=8))
        out_pool = ctx.enter_context(tc.tile_pool(name="outp", bufs=3))
        psum_pool = ctx.enter_context(
            tc.tile_pool(name="psum", bufs=8 // nbank, space="PSUM")
        )

        toep = const_pool.tile([128, 32 * N_TOEP], BF16)
        nc.sync.dma_start(out=toep[:, :], in_=toep_h.ap())
        cvec = const_pool.tile([128, 1], F32)
        nc.sync.dma_start(out=cvec[:, :], in_=cvec_h.ap())

        nhalves = ntiles * 2
        nats = [None] * nhalves

        N_RAW = 2  # trailing half-tiles loaded raw f32 over HWDGE

        def emit_input(t):
            g, h = divmod(t, 2)
            base = g * 128 * U + h * H  # flat sample offset of half-tile
            nat = in_pool.tile([128, H + HALO], BF16, tag="nat", name=f"nat{t}")
            H2 = H // 2
            if t >= nhalves - N_RAW:
                # last tiles: raw f32 via the otherwise-idle HWDGE ring so
                # their data is resident long before the SWDGE cast stream
                # finishes; ScalarE casts to bf16 (it has slack pre-squares)
                raw = raw_pool.tile([128, H + HALO], F32, tag="natr")
                if h == 0:
                    halo_view = bass.AP(
                        halos_h, g * HALO, [[ntiles * HALO, 128], [1, HALO]]
                    )
                    nc.scalar.dma_start(out=raw[:, 0:HALO], in_=halo_view)
                    main_view = bass.AP(s_h, base, [[U, 128], [1, H]])
                    nc.scalar.dma_start(out=raw[:, HALO:], in_=main_view)
                else:
                    ext_view = bass.AP(
                        s_h, base - HALO, [[U, 128], [1, H + HALO]]
                    )
                    nc.scalar.dma_start(out=raw[:, :], in_=ext_view)
                nc.scalar.copy(nat[:, :], raw[:, :])
                nats[t] = nat
                return
            if h == 0:
                # halo ghost cells, host-assembled during sharding (zeros on
                # row-start partitions, previous partition's tail elsewhere)
                halo_view = bass.AP(
                    halos_h, g * HALO, [[ntiles * HALO, 128], [1, HALO]]
                )
                nc.gpsimd.dma_start(out=nat[:, 0:HALO], in_=halo_view)
                if t == 0:
                    main_view = bass.AP(s_h, 0, [[U, 128], [1, H2]])
                    nc.gpsimd.dma_start(
                        out=nat[:, HALO : HALO + H2], in_=main_view
                    )
                    main2_view = bass.AP(s_h, H2, [[U, 128], [1, H2]])
                    nc.gpsimd.dma_start(
                        out=nat[:, HALO + H2 :], in_=main2_view
                    )
                else:
                    main_view = bass.AP(s_h, base, [[U, 128], [1, H]])
                    nc.gpsimd.dma_start(out=nat[:, HALO:], in_=main_view)
            elif t == 1:
                e1 = bass.AP(s_h, base - HALO, [[U, 128], [1, H2 + HALO]])
                nc.gpsimd.dma_start(out=nat[:, : H2 + HALO], in_=e1)
                e2 = bass.AP(s_h, base + H2, [[U, 128], [1, H2]])
                nc.gpsimd.dma_start(out=nat[:, H2 + HALO :], in_=e2)
            else:
                ext_view = bass.AP(
                    s_h, base - HALO, [[U, 128], [1, H + HALO]]
                )
                nc.gpsimd.dma_start(out=nat[:, :], in_=ext_view)
            nats[t] = nat

        # issue every input DMA up-front: the HWDGE queue stays deep so the
        # input stream never starves between half-tiles
        for t in range(nhalves):
            emit_input(t)

        sqs = [None] * nhalves
        for g in range(ntiles):
            for h in range(2):
                t = g * 2 + h
                nat = nats[t]
                st = st_pool.tile([128, H + HALO], BF16, tag="st")
                if t == 0:
                    # split the head-of-pipeline transpose so compute can
                    # start as soon as the first half of input 0 lands
                    H2 = H // 2
                    nc.vector.transpose(
                        st[:, : H2 + HALO], nat[:, : H2 + HALO]
                    )
                    nc.vector.transpose(
                        st[:, H2 + HALO :], nat[:, H2 + HALO :]
                    )
                else:
                    nc.vector.transpose(st[:, :], nat[:, :])

                sq = sq_pool.tile([128, H], BF16, tag="sq")
                sqs[t] = sq
                for w in range(nwin):
                    q = psum_pool.tile([128, W], F32, tag="q")
                    for sw in range(nbank):
                        c0 = (W // 32) * w + 16 * sw  # superblock of bank
                        for kidx in range(N_TOEP):
                            dlt = -kidx
                            s0 = 32 * (c0 + 1 + dlt)
                            for i in range(4):
                                pr = slice(32 * i, 32 * i + 32)
                                nc.tensor.matmul(
                                    q[pr, 512 * sw : 512 * sw + 512],
                                    toep[pr, 32 * kidx : 32 * kidx + 32],
                                    st[pr, s0 : s0 + 512],
                                    start=kidx == 0,
                                    stop=kidx == N_TOEP - 1,
                                    tile_position=(32 * i, 32 * i),
                                    skip_group_check=True,
                                )
                    nc.scalar.activation(
                        sq[:, w * W : (w + 1) * W],
                        q[:, :],
                        mybir.ActivationFunctionType.Square,
                    )

        # epilogue per half-tile, emitted as a separate block after every
        # forward transpose so DVE's in-order stream is never blocked by a
        # downstream dependency
        for t in range(nhalves):
            aff = out_pool.tile([128, H], BF16, tag="aff")
            eng = nc.vector
            eng.tensor_scalar(
                aff[:, :],
                sqs[t][:, :],
                -1.0,
                cvec[:, :],
                op0=mybir.AluOpType.mult,
                op1=mybir.AluOpType.add,
            )
            out_view = bass.AP(
                out_h, t * H, [[nhalves * H, 128], [1, H]]
            )
            nc.sync.dma_start(out=out_view, in_=aff[:, :])

    nc.compile()
    return nc


def make_consts(coeffs: np.ndarray, noise_std: float):
    """Host-side O(1) prep: banded Toeplitz filter matrices + constants."""
    coeffs = np.asarray(coeffs, dtype=np.float64).reshape(-1)
    p = coeffs.shape[0]
    sigma = float(noise_std)
    invsc = 1.0 / (math.sqrt(2.0) * sigma)
    c_const = -0.5 * math.log(2.0 * math.pi * sigma * sigma)
    h = np.zeros(p + 1, dtype=np.float64)
    h[0] = -invsc
    h[1:] = invsc * coeffs

    mats = []
    for dlt in (0, -1):
        # out sample 32C + m takes input sample 32(C+dlt) + k:
        # tap lag = m - k - 32*dlt
        T = np.zeros((32, 32), dtype=np.float64)
        for k in range(32):
            for m in range(32):
                lag = m - k - 32 * dlt
                if 0 <= lag <= p:
                    T[k, m] = h[lag]
        mats.append(T)

    import ml_dtypes

    toep = np.concatenate(mats, axis=1)                     # [32, 64]
    toep = np.tile(toep, (4, 1)).astype(ml_dtypes.bfloat16)  # [128, 64]
    cvec = np.full((128, 1), c_const, dtype=np.float32)
    return toep, cvec


def make_halos(s_core: np.ndarray, rows_per_tile: int) -> np.ndarray:
    """Ghost cells for each full tile: [128, ntiles*HALO] f32 where
    partition p's strip is the previous partition's tail (same row) or
    zeros at row starts."""
    b_core, t_len = s_core.shape
    R = rows_per_tile
    U = R * t_len // 128
    cpr = 128 // R
    ntiles = b_core // R
    sflat = np.ascontiguousarray(s_core).reshape(ntiles, 128, U)
    tails = sflat[:, :, U - HALO :]                      # [g, p, HALO]
    halos = np.zeros((ntiles, 128, HALO), dtype=np.float32)
    halos[:, 1:, :] = tails[:, :-1, :]
    halos[:, ::cpr, :] = 0.0
    return np.ascontiguousarray(halos.transpose(1, 0, 2)).reshape(
        128, ntiles * HALO
    )


def unshard_core(arr: np.ndarray, b_core: int, t_len: int,
                 rows_per_tile: int) -> np.ndarray:
    """De-interleave one core's [128, b_core*t_len/128] block-transposed
    output back to [b_core, t_len]. Pure reshape/transpose."""
    R = rows_per_tile
    U = R * t_len // 128
    H = U // 2
    ntiles = b_core // R
    cpr = 128 // R
    # arr[32i + m, ((g*2+h)*H + 32C + y)] = value(stream 32i+y, g, h, 32C+m)
    # stream p' = 32i+y -> row 8g + p'//cpr, seg p'%cpr
    # Only valid for R=8, cpr=16 (y in [0,32) -> yh=y//16, ys=y%16).
    assert R == 8 and cpr == 16
    A = arr.reshape(4, 32, ntiles, 2, H // 32, 32)  # i, m, g, h, C, y
    A = A.reshape(4, 32, ntiles, 2, H // 32, 2, 16)  # i, m, g, h, C, yh, ys
    O = A.transpose(2, 0, 5, 6, 3, 4, 1)  # g, i, yh, ys, h, C, m
    return np.ascontiguousarray(O).reshape(b_core, t_len)


_NC_CACHE: dict = {}


def _get_nc(b_core, t_len, rows_per_tile=8, win=1024):
    key = (b_core, t_len, rows_per_tile, win)
    if key not in _NC_CACHE:
        _NC_CACHE[key] = build_nc(b_core, t_len, rows_per_tile, win)
    return _NC_CACHE[key]


def run_on_hw(s, coeffs, noise_std, rows_per_tile=8, win=1024, trace=False,
              tmpdir=None):
    """Shard across 8 cores, run, gather. Returns (out, BassKernelResults)."""
    s = np.ascontiguousarray(np.asarray(s, dtype=np.float32))
    b_full, t_len = s.shape
    b_core = b_full // N_CORES
    nc = _get_nc(b_core, t_len, rows_per_tile, win)
    toep, cvec = make_consts(coeffs, float(np.asarray(noise_std)))
    in_maps = [
        {
            "s": s[i * b_core : (i + 1) * b_core],
            "toep": toep,
            "cvec": cvec,
            "halos": make_halos(s[i * b_core : (i + 1) * b_core],
                                rows_per_tile),
        }
        for i in range(N_CORES)
    ]
    res = run_bass_kernel_spmd(
        nc, in_maps, core_ids=list(range(N_CORES)), trace=trace, tmpdir=tmpdir
    )
    out = np.concatenate(
        [
            unshard_core(
                np.asarray(res.results[i]["out"], dtype=np.float32),
                b_core, t_len, rows_per_tile,
            )
            for i in range(N_CORES)
        ],
        axis=0,
    )
    return out, res


def kernel(s, coeffs, noise_std):
    out, _ = run_on_hw(s, coeffs, noise_std)
    return out

